# revision 80
# baseline (speedup 1.0000x reference)
"""Trainium2 Bass kernel for nn_Attention3D (RMSNorm3D + 1x1x1 QKV conv +
4-head non-flash attention over n=4096 tokens + 1x1x1 output conv).

Sharding: b*heads = 2*4 = 8 independent attention instances -> one per
NeuronCore. Per core: full [4096, 4096] score matrix for one (batch, head).

Key design (cost-model driven; ~108.6us vs 183us baseline):
  - x, wqkv in bf16. q/k/v projected token-major ([128t, 96]) on PE, scaled
    by the RMS-norm reciprocal (per-token = per-partition, DVE) and
    converted to fp8e4m3 in one tensor_scalar op.
  - q,k transposed (PE, fp8, strided element-step-2 psum writes) to
    dh-major and packed into one [32, 2, 8192] tile (q | k along columns)
    whose second k-tile is zeros, so QK runs as fp8 DoubleRow matmuls
    (0.5 cy/col): scoresT [128j, 512i].
  - exp split 50/50 between the Scalar engine (native Exp, scale=1/16384)
    and a custom 8-stage DVE op computing (1 + c0 x + c1 x^2)^8 ~ e^x
    (0.17% max rel err on |x|<=1.35); both read PSUM fp32 and write fp8
    e-tiles. Strict group alternation keeps both engines in lockstep.
  - PV: fp8 DoubleRow matmuls, lhsT = [v | ones] for 2 j-chunks
    ([128, 2, 33], 112-byte row stride for the 16B dual-fp8 alignment),
    accumulating numerator + softmax denominator in one pass.
  - output conv [33,65] @ num (unit row passes the denominator through);
    per-token division + head-sum + bias on host (they commute with the
    per-token scalar divide).
  - GPSIMD cannot read PSUM: all psum->sbuf traffic lives on DVE/Act.
  - phase 1 software-pipelines the projection with i-blocks 0 AND 1
    (interleaved), deferring their PV matmuls (deep e-tile pool) so the
    "m" psum slots stay free for the projection pipeline.
  - fp8 ranges centered with q*=256, k*=64, v*=64; exp scale 1/16384 in
    the activation scale / poly constants; 1/64 folded into w_out.
"""

import numpy as np

import concourse.bass as bass
import concourse.mybir as mybir
import concourse.tile as tile
import concourse.dve_ops as dve_ops
from concourse import bacc
from concourse.bass import ts
from concourse.bass_utils import run_bass_kernel_spmd
from concourse.dve_spec import C0, C1, One, Spec, Src0, lower, sq
from concourse.dve_uop import DveOpSpec
from concourse.masks import make_identity

# Initialize the PJRT backend immediately: the axon client handshake is
# flaky when the first device access happens long after process start.
try:
    import jax as _jax

    _jax.devices()
except Exception:
    pass

F32 = mybir.dt.float32
F32R = mybir.dt.float32r
BF16 = mybir.dt.bfloat16
FP8 = mybir.dt.float8e4

B = 2
C = 64
SP = (16, 16, 16)
N = 4096
HEADS = 4
DH = 32
HID = HEADS * DH
NC128 = N // 128
NIB = N // 512
EPS = 1e-12

ALPHA = 256.0  # q fp8 pre-scale
BETA = 64.0    # k fp8 pre-scale
GAMMA = 64.0   # v fp8 pre-scale
SCL = 1.0 / (ALPHA * BETA)  # exp input scale
# minimax fit of (1 + c0 x + c1 x^2)^8 ~ e^x over [-1.35, 1.35] (~0.17% max)
EC0 = 0.12543408184710148
EC1 = 0.0078111557515800276

ActF = mybir.ActivationFunctionType
DR = mybir.MatmulPerfMode.DoubleRow


def _register_exp_op():
    name = "EXP_POLY8_ANT"
    for op in dve_ops.OPS:
        if op.name == name:
            return op
    u = Src0 * C0
    x2 = Src0 * Src0
    v = x2 * C1
    b = (u + v) + One
    body = sq(sq(sq(b)))

    def ref(in0, in1, c0, c1, c2):
        xf = in0.astype(np.float32)
        bb = 1.0 + xf * np.float32(c0) + (xf * xf) * np.float32(c1)
        return (bb ** 8).astype(np.float32)

    spec = Spec(body=body, reference=ref)
    opcode = dve_ops._CUSTOM_DVE_ROW_BASE + len(dve_ops.OPS)
    shas = {}
    for ver in ("v3", "v4"):
        try:
            s = DveOpSpec(
                name=name, opcode=opcode, uops=lower(spec, ver=ver), rd1_en=False
            )
            shas[ver] = s.sha(ver)
        except Exception:
            pass
    op = dve_ops.DveOp(name, spec, subdim=False, uops_sha=shas)
    dve_ops.OPS.append(op)
    dve_ops._SUB_OPCODE_FOR_NAME[name] = opcode
    dve_ops.CUSTOM_DVE_SPECS[name] = spec
    return op


EXP_OP = _register_exp_op()


def build_nc():
    nc = bacc.Bacc("TRN2", target_bir_lowering=False, debug=False)

    xb = nc.dram_tensor("xb", [C, N], BF16, kind="ExternalInput")
    wqkv = nc.dram_tensor("wqkv", [C, 3 * DH], BF16, kind="ExternalInput")
    woT = nc.dram_tensor("woT", [DH + 1, C + 1], F32, kind="ExternalInput")
    zf8 = nc.dram_tensor("zf8", [DH, 2 * N], FP8, kind="ExternalInput")
    out_h = nc.dram_tensor("out_h", [C + 1, N], F32, kind="ExternalOutput")

    with tile.TileContext(nc) as tc:
        _body(tc, nc, xb, wqkv, woT, zf8, out_h)
    nc.compile()
    return nc


def _body(tc, nc, xb, wqkv, woT, zf8, out_h):
    const = tc.alloc_tile_pool(name="const", bufs=1)
    work = tc.alloc_tile_pool(name="work", bufs=2)
    epool = tc.alloc_tile_pool(name="epool", bufs=36)
    outp = tc.alloc_tile_pool(name="outp", bufs=2)
    # PSUM: exp/score tiles 3x[128,1024]f32 (6 banks, also borrowed by the
    # projection phase for its fp8 transpose collects) + misc 2x[128,512]f32
    # (2 banks: ss/proj/pv/out rotation) = 8 banks.
    ps_exp = tc.alloc_tile_pool(name="ps_exp", bufs=3, space="PSUM")
    ps_misc = tc.alloc_tile_pool(name="ps_misc", bufs=2, space="PSUM")

    # ---- constants / inputs ----
    id8 = const.tile([128, 128], FP8, name="id8")
    make_identity(nc, id8)
    ones64 = const.tile([C, 1], BF16, name="ones64")
    nc.gpsimd.memset(ones64, 1.0)

    x_sb = const.tile([C, N], BF16, name="x_sb")
    for i in range(4):
        nc.sync.dma_start(out=x_sb[:, ts(i, N // 4)], in_=xb[:, ts(i, N // 4)])
    qk4 = const.tile([DH, 2, 2 * N], FP8, name="qk4")
    nc.sync.dma_start(out=qk4[:, 1, :], in_=zf8[:, :])
    wqkv_sb = const.tile([C, 3 * DH], BF16, name="wqkv_sb")
    nc.sync.dma_start(out=wqkv_sb, in_=wqkv[:, :])
    # [33, 65]: rows 0:32 = w_out^T/GAMMA, row 32 = e_64 (passes the softmax
    # denominator through the output-conv matmul as out row 64)
    woT_sb = const.tile([DH + 1, C + 1], F32, name="woT_sb")
    nc.sync.dma_start(out=woT_sb, in_=woT[:, :])

    # qkv token-major fp8: [:, c, 0:32]=q, 32:64=k, 64:96=v, 96=ones
    # innermost dim padded to 112 (16B-aligned ktile step for dual-fp8 PV)
    qkv_sb = const.tile([128, NC128, 112], FP8, name="qkv_sb")
    nc.gpsimd.memset(qkv_sb[:, :, 3 * DH : 3 * DH + 1], 1.0)

    # ---- RMS norm: per-token reciprocal of l2 over channels ----
    # x^2 split across Pool/DVE/Act so the norm prefix is short.
    ps_ss = ps_misc.tile([128, 512], F32, tag="m", name="ps_ss")
    for g8 in range(4):
        xsq = outp.tile([C, 1024], BF16, tag="xsq", bufs=3)
        xs = x_sb[:, ts(g8, 1024)]
        nc.vector.tensor_mul(xsq, xs, xs)
        for l in range(8):
            c = 8 * g8 + l
            nc.tensor.matmul(
                ps_ss[:, c : c + 1], xsq[:, ts(l, 128)], ones64,
                start=True, stop=True,
            )
    l2t = work.tile([128, NC128], F32, tag="l2")
    nc.scalar.activation(l2t, ps_ss[:, 0:NC128], ActF.Sqrt)
    nc.vector.tensor_scalar_max(out=l2t, in0=l2t, scalar1=EPS)
    invT = const.tile([128, NC128], F32, name="invT")
    nc.vector.reciprocal(invT, l2t)

    def proj_part(g, part, state):
        """Emit one quarter of proj group g: 0=matmuls, 1=scales,
        2=transposes, 3=collect. Finer emission interleaves the pieces
        between attention steps so they pipeline inside the engine queues."""
        if part == 0:
            psp = ps_misc.tile([128, 512], F32, tag="m", name="psp")
            state[g] = psp
            for l in range(4):
                c = 4 * g + l
                nc.tensor.matmul(
                    psp[:, 96 * l : 96 * (l + 1)],
                    x_sb[:, ts(c, 128)], wqkv_sb,
                    start=True, stop=True,
                )
        elif part == 1:
            psp = state[g]
            for l in range(4):
                c = 4 * g + l
                nc.vector.tensor_scalar_mul(
                    out=qkv_sb[:, c, 0 : 3 * DH],
                    in0=psp[:, 96 * l : 96 * (l + 1)],
                    scalar1=invT[:, c : c + 1],
                )
        elif part == 2:
            # fp8 PE transposes write with element step 2, 4B-aligned
            # starts: q strided in bytes [0,1024), k in [1024,2048).
            pcol = ps_misc.tile([32, 2048], FP8, tag="m", name="pcol")
            state[(g, "pc")] = pcol
            pcr = pcol.rearrange("p (a n two) -> p a n two", a=2, two=2)
            for l in range(4):
                c = 4 * g + l
                nc.tensor.transpose(
                    pcr[0:DH, 0, 128 * l : 128 * (l + 1), 0:1],
                    qkv_sb[:, c, 0:DH], id8,
                )
                nc.tensor.transpose(
                    pcr[0:DH, 1, 128 * l : 128 * (l + 1), 0:1],
                    qkv_sb[:, c, DH : 2 * DH], id8,
                )
        else:
            pcol = state.pop((g, "pc"))
            state.pop(g, None)
            dst = qk4[0:DH, 0, :].rearrange("p (a b) -> p a b", a=2)[
                :, :, ts(g, 512)
            ]
            src = pcol[0:DH, :].rearrange(
                "p (a n two) -> p a n two", a=2, two=2
            )[:, :, :, 0]
            nc.scalar.copy(dst, src)

    # ---- attention: QK (fp8 DR) -> exp (Act/DVE) -> PV (fp8 DR) ----
    # ib 0 is software-pipelined with the projection phase: proj group g
    # produces exactly the k-chunks consumed by score groups 2g, 2g+1.
    s0 = float(EC0 * SCL)
    s1 = float(EC1 * SCL * SCL)
    pv_lag = [10]

    def qk_exp_group(ib, jp, par):
        pse = ps_exp.tile([128, 2, 512], F32, tag="e", name="pse")
        for t in range(2):
            jc = 2 * jp + t
            nc.tensor.matmul(
                pse[:, t, :],
                qk4[:, :, N + 128 * jc : N + 128 * (jc + 1)],
                qk4[:, :, ts(ib, 512)],
                start=True, stop=True, perf_mode=DR,
            )
        et = epool.tile([128, 2, 512], FP8, tag="e", name="et")
        if par:
            nc.scalar.activation(et, pse, ActF.Exp, scale=SCL)
        else:
            nc.vector._custom_dve(EXP_OP, out=et, in0=pse, s0=s0, s1=s1)
        return et

    def pv_mm(pv, jp, et):
        nc.tensor.matmul(
            pv,
            qkv_sb[:, 2 * jp : 2 * jp + 2, 2 * DH : 3 * DH + 1],
            et,
            start=(jp == 0), stop=(jp == 15), perf_mode=DR,
        )

    def out_stage(ib, pv):
        """Output conv + denominator row -> one [65, 512] DMA per i-block.
        The last i-block runs in two pipelined 256-column halves to shorten
        the serial tail chain."""
        halves = 2 if ib == NIB - 1 else 1
        w = 512 // halves
        for h in range(halves):
            num = outp.tile([DH + 1, 512], F32, tag="num", bufs=3)
            nc.scalar.copy(num[:, 0:w], pv[:, h * w : (h + 1) * w])
            ps_o = ps_misc.tile([C + 1, 512], F32, tag="m", name="ps_o")
            nc.tensor.matmul(
                ps_o[:, 0:w], woT_sb, num[:, 0:w], start=True, stop=True
            )
            o_sb = outp.tile([C + 1, 512], F32, tag="o", bufs=3)
            nc.scalar.copy(o_sb[:, 0:w], ps_o[:, 0:w])
            nc.sync.dma_start(
                out=out_h[:, 512 * ib + h * w : 512 * ib + (h + 1) * w],
                in_=o_sb[:, 0:w],
            )

    # Flat loop over all 128 score groups: PV matmuls trail QK by PV_LAG
    # across i-block boundaries so the PE queue never stalls on an exp, and
    # the out-stage of i-block ib is emitted right after its last PV.
    # ib 0's PVs are deferred entirely (lag 16) because the "m" psum slots
    # belong to the proj pipeline until it finishes.
    pvs = {}  # ib -> pv psum tile
    pending = []  # (ib, jp, et)

    in_phase1 = [True]

    def flush(done):
        # pop at most 2 per call so deferred PVs drain smoothly
        for _ in range(64 if done else 2):
            if not pending:
                return
            lag = 32 if in_phase1[0] else pv_lag[0]
            if not done and len(pending) <= lag:
                return
            pib, pjp, pet = pending.pop(0)
            if pjp == 0:
                pvs[pib] = ps_misc.tile(
                    [DH + 1, 512], F32, tag="m", name="pv"
                )
            pv_mm(pvs[pib], pjp, pet)
            if pjp == 15:
                out_stage(pib, pvs.pop(pib))

    # phase 1: ib0 and ib1 interleaved with the projection groups (each proj
    # group feeds the j-chunks of FOUR interleaved score groups, so the
    # projection pipeline runs at half the exp rate and never starves it);
    # their PVs are deferred until the projection releases the "m" slots.
    pstate = {}
    for step in range(32):
        ib, jp = step % 2, step // 2
        if step == 0:
            for part in range(4):
                proj_part(0, part, pstate)
        if step % 4 == 0 and step // 4 + 1 < 8:
            for part in range(4):
                proj_part(step // 4 + 1, part, pstate)
        et = qk_exp_group(ib, jp, step % 2 == 1)
        pending.append((ib, jp, et))
        flush(False)
    in_phase1[0] = False
    for gp in range(32, 16 * NIB):
        ib, jp = divmod(gp, 16)
        if gp == 16 * NIB - 12:
            pv_lag[0] = 3  # drain the lag early so the tail chain is short
        et = qk_exp_group(ib, jp, gp % 2 == 1)
        pending.append((ib, jp, et))
        flush(False)
    flush(True)

    for p in (ps_misc, ps_exp, outp, epool, work, const):
        p.release()


_NC_CACHE = {}


def _get_nc():
    if "nc" not in _NC_CACHE:
        _NC_CACHE["nc"] = build_nc()
    return _NC_CACHE["nc"]


def make_in_maps(x, g, w_qkv, w_out):
    """Per-core inputs. Core id = 4*batch + head."""
    x = np.asarray(x, np.float32)
    g = np.asarray(g, np.float32).reshape(C)
    w_qkv = np.asarray(w_qkv, np.float32)
    w_out = np.asarray(w_out, np.float32)

    colscale = g * np.sqrt(C)
    wq = w_qkv[0:HID] * colscale[None, :] * (DH ** -0.5) * ALPHA
    wk = w_qkv[HID : 2 * HID] * colscale[None, :] * BETA
    wv = w_qkv[2 * HID : 3 * HID] * colscale[None, :] * GAMMA

    zf8 = np.zeros((DH, 2 * N), dtype=mybir.dt.np(FP8))

    in_maps = []
    for b in range(B):
        xbv = np.ascontiguousarray(x[b].reshape(C, N)).astype(mybir.dt.np(BF16))
        for h in range(HEADS):
            sl = slice(DH * h, DH * (h + 1))
            wqkv_core = np.ascontiguousarray(
                np.concatenate([wq[sl], wk[sl], wv[sl]], axis=0).T
            ).astype(mybir.dt.np(BF16))
            woT_core = np.zeros((DH + 1, C + 1), np.float32)
            woT_core[0:DH, 0:C] = (w_out[:, sl] / GAMMA).T
            woT_core[DH, C] = 1.0
            in_maps.append(
                {"xb": xbv, "wqkv": wqkv_core, "woT": woT_core, "zf8": zf8}
            )
    return in_maps


def kernel(x, g, w_qkv, w_out, b_out):
    nc = _get_nc()
    in_maps = make_in_maps(x, g, w_qkv, w_out)
    res = run_bass_kernel_spmd(nc, in_maps, core_ids=list(range(8)))
    b_out = np.asarray(b_out, np.float32)
    full = np.empty((B, C) + SP, np.float32)
    for b in range(B):
        acc = np.zeros((C, N), np.float32)
        for h in range(HEADS):
            oh = res.results[4 * b + h]["out_h"]
            acc += oh[0:C] / oh[C][None, :]
        full[b] = (acc + b_out[:, None]).reshape((C,) + SP)
    return full
